# revision 19
# baseline (speedup 1.0000x reference)
# Trainium2 Bass kernel for nn_Attention_45724221833993.
#
# Reference model (per batch b, modality m in {0,1}):
#   x_ma = PVT spatial-reduction attention over x_m (8x8/stride-8 conv keys, 2 heads)
#   s_m  = softmax_C(gelu(concat(x_ma, x_ob) @ w1.T + b1) @ w2.T + b2)
#   2-key per-token cross attention (8 heads) + residual + final projection
#
# Sharding: 8 cores = (batch 0..3) x (token half). The host rolls the inputs by
# 8192 tokens for second-half cores, so every core computes tokens [0:8192] of
# its (rolled) image. A 64-row roll of the 128x128 image permutes the 256
# stride-8 conv patches (64 is a multiple of the 8-row patch height), and
# attention is permutation-invariant over its keys, so results match the
# unrolled reference exactly.
#
# On-device layout is channel-major: activations are [C=128 partitions, tokens].
# Weights are pre-transposed/pre-composed on the host (layout-only transforms +
# O(C^3) compositions). All matmul operands are bf16 with fp32 PSUM
# accumulation; softmax/gelu/layernorm math is fp32 on ACT/DVE.
#
# Algebraic folds used:
#  - q-projection folded into keys: keff_h = Wq_h.T @ k_h.T, so scores come
#    straight from x^T; the q bias becomes a per-key multiplicative factor
#    e_k = exp(scale * k.bq) folded into V and the denominator weights.
#    e_k itself comes from 2 matmuls with a head-masked bq pair as the rhs.
#  - softmax denominators via ones-matmul broadcast (replicated across the 64
#    rows of each head) + reciprocal_approx_fast.
#  - layernorm over channels without transposes: ones-column matmul gives
#    per-token sum(x) and sum(x^2); rstd/mean*rstd are computed on one
#    partition and broadcast back with a ones-row matmul.
#  - judger gate enters only через kd: kd = -Wk.T @ ((s_m - 1) * x_ma) + nk,
#    with (s_m - 1) * x_ma computed as two elementwise ops on GpSimd.
#  - mha2 softmax over 2 keys: a0 = 0.5*(1 + tanh((s0-s1)/2)); s0-s1 computed
#    with a single difference-projection matmul (biases cancel).
#  - mha2 out + residual + final projection collapsed into one 3-term GEMM:
#    out = P@x_m + (P Wo Wv)@x_o + (P Wo)@(a0*dv) + const.
#
# ACT function-table phasing: conv-LN + attention only use {Square, Ln, Exp,
# Identity} (one table set), all judger gelus are batched into a dedicated
# phase (gelu set), and the remaining post ops only use {Exp, Tanh, Identity}
# (one set) -- 3 table loads total instead of ~39.

import numpy as np
import ml_dtypes
from contextlib import ExitStack

import concourse.bass as bass
import concourse.bacc as bacc
import concourse.tile as tile
from concourse.tile import add_dep_helper
from concourse import mybir
from concourse.bass_utils import run_bass_kernel_spmd

F32 = mybir.dt.float32
BF16 = mybir.dt.bfloat16
AF = mybir.ActivationFunctionType
ALU = mybir.AluOpType

B, HI, WI, C, HEADS, XHEADS, SR = 4, 128, 128, 128, 2, 8, 8
NIMG = HI * WI               # 16384 tokens per image
T = NIMG // 2                # 8192 tokens owned per core
M = (HI // SR) * (WI // SR)  # 256 conv patches (keys)
D = C // HEADS               # 64
DX = C // XHEADS             # 16
SCALE = D ** -0.5            # 0.125
XSCALE = DX ** -0.5          # 0.25
NCH = T // 512               # 16
NG = T // 1024               # 8 post-phase groups per modality
LN_EPS = 1e-5
INV_C = 1.0 / C

bf16 = ml_dtypes.bfloat16

WEIGHT_NAMES_BF16 = (
    ["wkvT0", "wkvT1", "wq", "bq2", "ident", "ones64", "ones128",
     "w1aT", "w1bT", "w2T", "ind8", "indB4", "pT"]
    + [f"{n}{m}" for m in range(2)
       for n in ("wqxT", "nwkxT", "wvxT", "w2fT", "w3fT")]
)
WEIGHT_NAMES_F32 = (
    ["srb_col", "b1_col", "b2_col", "bkv_col0", "bkv_col1"]
    + [f"{n}{m}" for m in range(2)
       for n in ("bqx_col", "nk_col", "nvh_col", "cfin_col")]
)
WEIGHT_SHAPES = {
    "srwT": (C, SR * SR, C), "wkvT0": (C, 2 * C), "wkvT1": (C, 2 * C),
    "wq": (C, C), "bq2": (C, 2), "ident": (C, C),
    "ones64": (C, 64), "ones128": (C, C),
    "w1aT": (C, C), "w1bT": (C, C), "w2T": (C, C),
    "ind8": (C, 32), "indB4": (C, C), "pT": (C, C),
    "srb_col": (C, 1), "b1_col": (C, 1), "b2_col": (C, 1),
    "bkv_col0": (C, 2), "bkv_col1": (C, 2),
}
for _m in range(2):
    for _n in ("wqxT", "nwkxT", "wvxT", "w2fT", "w3fT"):
        WEIGHT_SHAPES[f"{_n}{_m}"] = (C, C)
    for _n in ("bqx_col", "nk_col", "nvh_col", "cfin_col"):
        WEIGHT_SHAPES[f"{_n}{_m}"] = (C, 1)


# ---------------------------------------------------------------------------
# bass program
# ---------------------------------------------------------------------------

def build_nc():
    nc = bacc.Bacc(trn_type="TRN2")

    di = {}
    for m in range(2):
        di[f"xT{m}"] = nc.dram_tensor(f"xT{m}", [C, NIMG], BF16,
                                      kind="ExternalInput").ap()
    di["srwT"] = nc.dram_tensor("srwT", [C, SR * SR, C], BF16,
                                kind="ExternalInput").ap()
    nb = sum(WEIGHT_SHAPES[n][1] for n in WEIGHT_NAMES_BF16)
    nf = sum(WEIGHT_SHAPES[n][1] for n in WEIGHT_NAMES_F32)
    di["wpackB"] = nc.dram_tensor("wpackB", [C, nb], BF16,
                                  kind="ExternalInput").ap()
    di["wpackF"] = nc.dram_tensor("wpackF", [C, nf], F32,
                                  kind="ExternalInput").ap()
    out = nc.dram_tensor("out", [2, C, T], BF16, kind="ExternalOutput").ap()

    with ExitStack() as ctx:
        tc = ctx.enter_context(tile.TileContext(nc))

        wpool = ctx.enter_context(tc.tile_pool(name="weights", bufs=1))
        apool = ctx.enter_context(tc.tile_pool(name="xa", bufs=1))
        psA = ctx.enter_context(tc.tile_pool(name="psA", bufs=3, space="PSUM"))
        psB = ctx.enter_context(tc.tile_pool(name="psB", bufs=2, space="PSUM"))
        # phase-scoped sbuf pools: xt+attn work (incl. the conv weights and
        # all kv-phase scratch) close before the post phase opens, so their
        # 100+ KB/partition is reused.
        phase1 = ExitStack()
        xpool = phase1.enter_context(tc.tile_pool(name="xt", bufs=1))
        small = phase1.enter_context(tc.tile_pool(name="small", bufs=2))
        watt = phase1.enter_context(tc.tile_pool(name="watt", bufs=3))

        # input DMAs; xT0 first so conv m=0 can start as early as possible.
        # Spread across different engines' DGE queues so they overlap.
        xT = {}
        tl0 = xpool.tile([C, NIMG], BF16, name="xT0", tag="xT0")
        nc.sync.dma_start(out=tl0[:, :T], in_=di["xT0"][:, :T])
        nc.scalar.dma_start(out=tl0[:, T:], in_=di["xT0"][:, T:])
        xT[0] = tl0

        w = {}
        tl = xpool.tile([C, SR * SR, C], BF16, name="w_srwT", tag="w_srwT")
        nc.gpsimd.dma_start(out=tl, in_=di["srwT"])
        w["srwT"] = tl
        wpB = wpool.tile([C, nb], BF16, name="wpackB", tag="wpackB")
        nc.gpsimd.dma_start(out=wpB, in_=di["wpackB"])
        wpF = wpool.tile([C, nf], F32, name="wpackF", tag="wpackF")
        nc.gpsimd.dma_start(out=wpF, in_=di["wpackF"])

        tl1 = xpool.tile([C, NIMG], BF16, name="xT1", tag="xT1")
        nc.sync.dma_start(out=tl1[:, :T], in_=di["xT1"][:, :T])
        nc.scalar.dma_start(out=tl1[:, T:], in_=di["xT1"][:, T:])
        xT[1] = tl1

        off = 0
        for name in WEIGHT_NAMES_BF16:
            k = WEIGHT_SHAPES[name][1]
            w[name] = wpB[:, off:off + k]
            off += k
        off = 0
        for name in WEIGHT_NAMES_F32:
            k = WEIGHT_SHAPES[name][1]
            w[name] = wpF[:, off:off + k]
            off += k

        eps_col = wpool.tile([C, 1], F32, name="eps_col", tag="eps_col")
        nc.vector.memset(eps_col, LN_EPS)
        ones_col = wpool.tile([C, 1], F32, name="ones_col", tag="ones_col")
        nc.vector.memset(ones_col, 1.0)
        ones_row = wpool.tile([1, C], F32, name="ones_row", tag="ones_row")
        nc.vector.memset(ones_row, 1.0)

        xa = {m: apool.tile([C, T], BF16, name=f"xa{m}", tag=f"xa{m}")
              for m in range(2)}

        # =================================================================
        # Phase KV: conv -> LN(channel) -> kv -> keff / V' / e-scaled ones
        # The Ln activations of both modalities are batched back-to-back so
        # the act-table pass emits one natural_log load instead of two
        # ln<->exp bounces.
        # =================================================================
        keff, vext, onese = {}, {}, {}
        st_t, sm_t, var_t, rm_t = {}, {}, {}, {}
        for m in range(2):
            ps_conv = psB.tile([C, M], F32, name=f"conv{m}", tag="B")
            lat = xT[m].rearrange("c (pr i pc j) -> c i j pr pc",
                                  pr=16, i=8, pc=16, j=8)
            for ij in range(SR * SR):
                i, j = ij // SR, ij % SR
                nc.tensor.matmul(ps_conv, w["srwT"][:, ij], lat[:, i, j],
                                 start=(ij == 0), stop=(ij == SR * SR - 1))
            # st = [xi | xi^2] side by side for the one-matmul LN stats
            st = small.tile([C, 2, M], F32, name=f"st{m}", tag=f"st{m}")
            nc.vector.tensor_scalar_add(st[:, 0], ps_conv, w["srb_col"])
            nc.scalar.activation(st[:, 1], ps_conv, AF.Square,
                                 bias=w["srb_col"], scale=1.0)
            ps_st = psB.tile([1, 2, M], F32, name=f"lns{m}", tag="B")
            nc.tensor.matmul(ps_st, ones_col, st, start=True, stop=True)
            # per-token stats on partition 0: mean | mean(x^2)
            sm = small.tile([1, 2, M], F32, name=f"sm{m}", tag=f"sm{m}")
            nc.vector.tensor_scalar_mul(sm, ps_st, INV_C)
            var = small.tile([1, 2, M], F32, name=f"var{m}", tag=f"var{m}")
            nc.vector.tensor_tensor(out=var[:, 0], in0=sm[:, 0], in1=sm[:, 0],
                                    op=ALU.mult)
            nc.vector.tensor_tensor(out=var[:, 1], in0=sm[:, 1], in1=var[:, 0],
                                    op=ALU.subtract)
            st_t[m], sm_t[m], var_t[m] = st, sm, var

        # fences keep the two Ln activations adjacent in the ACT stream so
        # the act-table pass emits one natural_log load, not two bounces
        tc.no_sync_barrier()
        for m in range(2):
            rm = small.tile([1, 2, M], F32, name=f"rm{m}", tag=f"rm{m}")
            nc.scalar.activation(rm[:, 0], var_t[m][:, 1], AF.Ln,
                                 bias=eps_col[0:1], scale=1.0)
            rm_t[m] = rm
        tc.no_sync_barrier()

        for m in range(2):
            st, sm, rm = st_t[m], sm_t[m], rm_t[m]
            nc.scalar.activation(rm[:, 0], rm[:, 0], AF.Exp,
                                 bias=0.0, scale=-0.5)
            nc.vector.tensor_tensor(out=rm[:, 1], in0=sm[:, 0], in1=rm[:, 0],
                                    op=ALU.mult)
            # broadcast rstd | mean*rstd to all 128 partitions
            ps_b = psB.tile([C, 2, M], F32, name=f"lnb{m}", tag="B")
            nc.tensor.matmul(ps_b, ones_row, rm, start=True, stop=True)
            zt1 = small.tile([C, M], F32, name=f"zt1{m}", tag="zt1")
            nc.vector.tensor_tensor(out=zt1, in0=st[:, 0], in1=ps_b[:, 0],
                                    op=ALU.mult)
            zT = small.tile([C, M], BF16, name=f"zT{m}", tag="zT")
            nc.vector.tensor_tensor(out=zT, in0=zt1, in1=ps_b[:, 1],
                                    op=ALU.subtract)

            # kv projection (k rows then v rows), fused LN-affine in weights
            k_sb = small.tile([C, M], BF16, name=f"k{m}", tag="ksb")
            v_sb = small.tile([C, M], BF16, name=f"v{m}", tag="vsb")
            for kv_i, dst in ((0, k_sb), (1, v_sb)):
                ps_kv = psB.tile([C, M], F32, name=f"kv{m}{kv_i}", tag="B")
                nc.tensor.matmul(ps_kv, w[f"wkvT{m}"][:, kv_i * C:(kv_i + 1) * C],
                                 zT, start=True, stop=True)
                nc.vector.tensor_scalar_add(dst, ps_kv,
                                            w[f"bkv_col{m}"][:, kv_i:kv_i + 1])

            # per-key factors e_k = exp(scale * k_h . bq_h) via head-masked bq
            ps_kb = psB.tile([C, 2, 2], F32, name=f"kb{m}", tag="B")
            for kt in range(2):
                nc.tensor.matmul(ps_kb[:, kt], k_sb[:, kt * C:(kt + 1) * C],
                                 w["bq2"], start=True, stop=True)
            e_sb = small.tile([C, 2, 2], F32, name=f"e{m}", tag="esb")
            nc.scalar.activation(e_sb, ps_kb, AF.Exp, bias=0.0, scale=SCALE)

            # keff_h.T = Wq_h.T @ k_h.T
            keff[m] = []
            for h in range(HEADS):
                hs = slice(h * D, (h + 1) * D)
                ps_ke = psB.tile([C, M], F32, name=f"ke{m}{h}", tag="B")
                nc.tensor.matmul(ps_ke, w["wq"][hs], k_sb[hs],
                                 start=True, stop=True)
                ke = small.tile([C, M], BF16, name=f"keff{m}{h}", tag=f"keff{h}")
                nc.vector.tensor_copy(ke, ps_ke)
                keff[m].append(ke)

            # V' = e-scaled values in [key, d] layout, one slab per (h, kt)
            ve = small.tile([C, 4, D], BF16, name=f"vext{m}", tag="vext")
            for hk in range(4):
                h, kt = hk // 2, hk % 2
                hs = slice(h * D, (h + 1) * D)
                ps_vt = psB.tile([C, D], BF16, name=f"vt{m}{hk}", tag="B")
                nc.tensor.transpose(ps_vt, v_sb[hs, kt * C:(kt + 1) * C],
                                    w["ident"][hs, hs])
                nc.vector.tensor_scalar_mul(ve[:, hk], ps_vt,
                                            e_sb[:, kt, h:h + 1])
            vext[m] = ve

            oe4 = small.tile([C, 4, 64], BF16, name=f"onese{m}", tag="onese")
            for hk in range(4):
                h, kt = hk // 2, hk % 2
                nc.vector.tensor_scalar_mul(oe4[:, hk], w["ones64"],
                                            e_sb[:, kt, h:h + 1])
            onese[m] = oe4

        # =================================================================
        # Phase ATTN: scores -> exp -> denominators -> AV -> normalize
        # =================================================================
        for m in range(2):
            for ch in range(NCH):
                ts = slice(ch * 512, (ch + 1) * 512)
                ps_sh = [psA.tile([C, 2, 512], F32, name=f"sc{m}{ch}{h}",
                                  tag="A") for h in range(2)]
                for hk in range(4):
                    h, kt = hk // 2, hk % 2
                    nc.tensor.matmul(ps_sh[h][:, kt],
                                     keff[m][h][:, kt * C:(kt + 1) * C],
                                     xT[m][:, ts], start=True, stop=True)
                expS = watt.tile([C, 4, 512], BF16, name=f"es{m}{ch}",
                                 tag="expS", bufs=2)
                for h in range(2):
                    attn_exp_last = nc.scalar.activation(
                        expS[:, 2 * h:2 * h + 2], ps_sh[h],
                        AF.Exp, bias=0.0, scale=SCALE)

                ps_den = psB.tile([C, 512], F32, name=f"den{m}{ch}", tag="B")
                for hk in range(4):
                    h, kt = hk // 2, hk % 2
                    nc.tensor.matmul(ps_den[h * 64:(h + 1) * 64, :],
                                     onese[m][:, hk], expS[:, hk],
                                     start=(kt == 0), stop=(kt == 1),
                                     tile_position=(0, h * 64))
                rden = watt.tile([C, 512], F32, name=f"rd{m}{ch}",
                                 tag="rden")
                nc.vector.reciprocal_approx_fast(out=rden, in_=ps_den)

                ps_av = psB.tile([C, 512], F32, name=f"av{m}{ch}", tag="B")
                for hk in range(4):
                    h, kt = hk // 2, hk % 2
                    nc.tensor.matmul(ps_av[h * 64:(h + 1) * 64, :],
                                     vext[m][:, hk], expS[:, hk],
                                     start=(kt == 0), stop=(kt == 1),
                                     tile_position=(0, h * 64))
                nc.vector.tensor_tensor(out=xa[m][:, ts], in0=ps_av, in1=rden,
                                        op=ALU.mult)

        # close phase-1 pools (xT + attn transients), open post-phase pools
        phase1.close()
        work = ctx.enter_context(tc.tile_pool(name="work", bufs=3))
        gpool = ctx.enter_context(tc.tile_pool(name="gelu", bufs=1))

        # =================================================================
        # xdiff = xa0 - xa1 (shared by both modalities' dv projections)
        # =================================================================
        xdiff = apool.tile([C, T], BF16, name="xdiff", tag="xdiff")
        for ch in range(NG):
            ts = slice(ch * 1024, (ch + 1) * 1024)
            nc.gpsimd.tensor_tensor(out=xdiff[:, ts], in0=xa[0][:, ts],
                                    in1=xa[1][:, ts], op=ALU.subtract)

        # =================================================================
        # Phase J-pre: all judger gelus. Ordering edges (not barriers) keep
        # the ACT stream clean: every gelu runs after the last attention
        # exp, so the act-table pass emits one gelu load. PE/DVE work is
        # free to flow across the phase boundary.
        # =================================================================
        g_sb = {}
        gelu_last = None
        for m in range(2):
            mo = 1 - m
            for g in range(NG):
                ps_h = psA.tile([C, 2, 512], F32, name=f"jh{m}{g}", tag="A")
                for q in range(2):
                    qs = slice(g * 1024 + q * 512, g * 1024 + (q + 1) * 512)
                    nc.tensor.matmul(ps_h[:, q], w["w1aT"], xa[m][:, qs],
                                     start=True, stop=False)
                    nc.tensor.matmul(ps_h[:, q], w["w1bT"], xa[mo][:, qs],
                                     start=False, stop=True)
                gt = gpool.tile([C, 1024], BF16, name=f"g{m}{g}",
                                tag=f"g{m}{g}")
                gelu_last = nc.scalar.activation(gt, ps_h, AF.Gelu,
                                                 bias=w["b1_col"], scale=1.0)
                add_dep_helper(gelu_last.ins, attn_exp_last.ins, sync=False,
                               reason="act-table: gelus after attention exps")
                g_sb[(m, g)] = gt

        # =================================================================
        # Phase J-pre2: judger softmax gate u = (s_m - 1) * xa for all
        # groups, pipelined ahead of the PE-dense J-post loop.
        # =================================================================
        u_sb = {}
        for m in range(2):
            for g in range(NG):
                gs = slice(g * 1024, (g + 1) * 1024)
                ps_l = psA.tile([C, 2, 512], F32, name=f"jl{m}{g}", tag="A")
                for q in range(2):
                    nc.tensor.matmul(ps_l[:, q], w["w2T"],
                                     g_sb[(m, g)][:, q * 512:(q + 1) * 512],
                                     start=True, stop=True)
                expL = work.tile([C, 2, 512], BF16, name=f"el{m}{g}",
                                 tag="expL", bufs=2)
                expl_inst = nc.scalar.activation(expL, ps_l, AF.Exp,
                                                 bias=w["b2_col"], scale=1.0)
                add_dep_helper(expl_inst.ins, gelu_last.ins, sync=False,
                               reason="act-table: post exps after gelus")

                ps_jd = psA.tile([C, 2, 512], F32, name=f"jd{m}{g}", tag="A")
                for q in range(2):
                    nc.tensor.matmul(ps_jd[:, q], w["ones128"], expL[:, q],
                                     start=True, stop=True)
                jrden = work.tile([C, 2, 512], F32, name=f"jr{m}{g}",
                                  tag="jrden", bufs=2)
                nc.vector.reciprocal_approx_fast(out=jrden, in_=ps_jd)

                t0 = work.tile([C, 1024], BF16, name=f"t0{m}{g}", tag="t0",
                               bufs=3)
                nc.gpsimd.tensor_tensor(out=t0, in0=expL, in1=jrden,
                                        op=ALU.mult)
                ut = gpool.tile([C, 1024], BF16, name=f"u{m}{g}",
                                tag=f"u{m}{g}")
                nc.vector.scalar_tensor_tensor(ut, t0, 1.0, xa[m][:, gs],
                                               op0=ALU.subtract, op1=ALU.mult)
                u_sb[(m, g)] = ut

        # =================================================================
        # Phase J-post: mha2 + residual + final projection (PE-dense)
        # =================================================================
        for m in range(2):
            mo = 1 - m
            for g in range(NG):
                gs = slice(g * 1024, (g + 1) * 1024)
                ps_sd = psB.tile([64, 512], F32, name=f"sd{m}{g}", tag="B")
                kd = {}
                for q in range(2):
                    qs = slice(g * 1024 + q * 512, g * 1024 + (q + 1) * 512)
                    ps_qpkd = psA.tile([C, 2, 512], F32, name=f"qpkd{m}{g}{q}",
                                       tag="A")
                    ps_qp, ps_kd = ps_qpkd[:, 0], ps_qpkd[:, 1]
                    nc.tensor.matmul(ps_qp, w[f"wqxT{m}"], xa[m][:, qs],
                                     start=True, stop=True)
                    nc.tensor.matmul(ps_kd, w[f"nwkxT{m}"],
                                     u_sb[(m, g)][:, q * 512:(q + 1) * 512],
                                     start=True, stop=True)
                    kdq = work.tile([C, 512], BF16, name=f"kds{m}{g}{q}",
                                    tag="kd", bufs=4)
                    nc.scalar.activation(kdq, ps_kd, AF.Identity,
                                         bias=w[f"nk_col{m}"], scale=1.0)
                    kd[q] = kdq

                    # qk = (qp_raw + bqx) * kd in one pass, qp read from PSUM
                    qk = work.tile([C, 512], BF16, name=f"qk{m}{g}{q}",
                                   tag="qk", bufs=4)
                    nc.vector.scalar_tensor_tensor(qk, ps_qp,
                                                   w[f"bqx_col{m}"], kdq,
                                                   op0=ALU.add, op1=ALU.mult)

                    # head-sum strip (rows 8:32 of ind8 are zero padding)
                    nc.tensor.matmul(ps_sd[32 * q:32 * (q + 1), :], w["ind8"],
                                     qk, start=True, stop=True,
                                     tile_position=(0, 32 * q))

                th = work.tile([64, 512], BF16, name=f"th{m}{g}", tag="tanh")
                nc.scalar.activation(th, ps_sd, AF.Tanh, bias=0.0, scale=0.5)

                o_sb = work.tile([C, 2, 512], BF16, name=f"o{m}{g}", tag="osb",
                                 bufs=2)
                for q in range(2):
                    qs = slice(g * 1024 + q * 512, g * 1024 + (q + 1) * 512)
                    ps_tbdv = psA.tile([C, 2, 512], F32, name=f"tbdv{m}{g}{q}",
                                       tag="A")
                    ps_tb, ps_dv = ps_tbdv[:, 0], ps_tbdv[:, 1]
                    nc.tensor.matmul(ps_tb, w["indB4"][32 * q:32 * q + XHEADS],
                                     th[32 * q:32 * q + XHEADS, :],
                                     start=True, stop=True,
                                     tile_position=(32 * q, 0))

                    nc.tensor.matmul(ps_dv, w[f"wvxT{m}"], xdiff[:, qs],
                                     start=True, stop=True)
                    dvh = work.tile([C, 512], BF16, name=f"dvh{m}{g}{q}",
                                    tag="dvh")
                    nc.scalar.activation(dvh, ps_dv, AF.Identity,
                                         bias=w[f"nvh_col{m}"], scale=0.5)
                    adv = work.tile([C, 512], BF16, name=f"adv{m}{g}{q}",
                                    tag="adv")
                    nc.vector.scalar_tensor_tensor(adv, ps_tb, 1.0, dvh,
                                                   op0=ALU.add, op1=ALU.mult)

                    ps_f = psB.tile([C, 512], F32, name=f"f{m}{g}{q}", tag="B")
                    nc.tensor.matmul(ps_f, w["pT"], xa[m][:, qs],
                                     start=True, stop=False)
                    nc.tensor.matmul(ps_f, w[f"w2fT{m}"], xa[mo][:, qs],
                                     start=False, stop=False)
                    nc.tensor.matmul(ps_f, w[f"w3fT{m}"], adv,
                                     start=False, stop=True)
                    nc.scalar.activation(o_sb[:, q], ps_f, AF.Identity,
                                         bias=w[f"cfin_col{m}"], scale=1.0)
                nc.sync.dma_start(out=out[m, :, gs], in_=o_sb)

    nc.compile()
    return nc


# ---------------------------------------------------------------------------
# host side
# ---------------------------------------------------------------------------

def _np(x):
    return np.asarray(x)


def prep_weights(i):
    """Host-side weight package: layout transforms and tiny O(C^3) composites."""
    f32 = np.float32
    Wq = _np(i["Wq"]).astype(f32)
    bq = _np(i["bq"]).astype(f32)
    Wkv = _np(i["Wkv"]).astype(f32)
    bkv = _np(i["bkv"]).astype(f32)
    sr_w = _np(i["sr_w"]).astype(f32)          # [co, ci, 8, 8]
    sr_b = _np(i["sr_b"]).astype(f32)
    ln_g = [_np(i["ln0_g"]).astype(f32), _np(i["ln1_g"]).astype(f32)]
    ln_b = [_np(i["ln0_b"]).astype(f32), _np(i["ln1_b"]).astype(f32)]
    w1 = _np(i["rj_w1"]).astype(f32)           # [C, 2C]
    b1 = _np(i["rj_b1"]).astype(f32)
    w2 = _np(i["rj_w2"]).astype(f32)
    b2 = _np(i["rj_b2"]).astype(f32)
    k_noise = _np(i["k_noise"]).astype(f32)
    v_noise = _np(i["v_noise"]).astype(f32)
    P = _np(i["proj_w"]).astype(f32)
    pb = _np(i["proj_b"]).astype(f32)

    pkg = {}

    def put(name, arr, dt=bf16):
        a = np.ascontiguousarray(np.asarray(arr, dtype=f32).astype(dt))
        assert a.shape == tuple(WEIGHT_SHAPES[name]), (name, a.shape)
        pkg[name] = a

    # [ci, ij, co] so the device DMA is fully contiguous per partition
    put("srwT", sr_w.transpose(1, 2, 3, 0).reshape(C, SR * SR, C))
    put("srb_col", sr_b.reshape(C, 1), f32)
    put("wq", Wq)
    bq2 = np.zeros((C, 2), f32)
    for h in range(HEADS):
        bq2[h * D:(h + 1) * D, h] = bq[h * D:(h + 1) * D]
    put("bq2", bq2)
    put("ident", np.eye(C, dtype=f32))
    put("ones64", np.ones((C, 64), f32))
    put("ones128", np.ones((C, C), f32))

    for m in range(2):
        weff = Wkv * ln_g[m][None, :]
        beff = Wkv @ ln_b[m] + bkv
        put(f"wkvT{m}", weff.T)
        put(f"bkv_col{m}", np.stack([beff[:C], beff[C:]], axis=1), f32)

    put("w1aT", w1[:, :C].T)
    put("w1bT", w1[:, C:].T)
    put("b1_col", b1.reshape(C, 1), f32)
    put("w2T", w2.T)
    put("b2_col", b2.reshape(C, 1), f32)

    ind8 = np.zeros((C, 32), f32)
    for h in range(XHEADS):
        ind8[h * DX:(h + 1) * DX, h] = XSCALE
    put("ind8", ind8)
    indB4 = np.zeros((C, C), f32)
    for base in (0, 32, 64, 96):
        for h in range(XHEADS):
            indB4[base + h, h * DX:(h + 1) * DX] = 1.0
    put("indB4", indB4)
    put("pT", P.T)

    ca = [(_np(i["ca01_in_w"]).astype(f32), _np(i["ca01_in_b"]).astype(f32),
           _np(i["ca01_out_w"]).astype(f32), _np(i["ca01_out_b"]).astype(f32)),
          (_np(i["ca10_in_w"]).astype(f32), _np(i["ca10_in_b"]).astype(f32),
           _np(i["ca10_out_w"]).astype(f32), _np(i["ca10_out_b"]).astype(f32))]
    for m in range(2):
        in_w, in_b, out_w, out_b = ca[m]
        Wqx, Wkx, Wvx = in_w[:C], in_w[C:2 * C], in_w[2 * C:]
        bqx, bkx, bvx = in_b[:C], in_b[C:2 * C], in_b[2 * C:]
        put(f"wqxT{m}", Wqx.T)
        put(f"bqx_col{m}", bqx.reshape(C, 1), f32)
        put(f"nwkxT{m}", -Wkx.T)
        put(f"nk_col{m}", (k_noise[m] @ Wkx.T).reshape(C, 1), f32)
        sgn = 1.0 if m == 0 else -1.0          # xdiff = xa0 - xa1 is shared
        put(f"wvxT{m}", sgn * Wvx.T)
        put(f"nvh_col{m}", (0.5 * (v_noise[m] @ Wvx.T)).reshape(C, 1), f32)
        PWo = P @ out_w
        put(f"w3fT{m}", PWo.T)
        put(f"w2fT{m}", (PWo @ Wvx).T)
        put(f"cfin_col{m}", (P @ out_b + pb + PWo @ bvx).reshape(C, 1), f32)

    packed = {"srwT": np.ascontiguousarray(pkg["srwT"])}
    packed["wpackB"] = np.ascontiguousarray(np.concatenate(
        [pkg[n].reshape(C, -1) for n in WEIGHT_NAMES_BF16], axis=1))
    packed["wpackF"] = np.ascontiguousarray(np.concatenate(
        [pkg[n].reshape(C, -1) for n in WEIGHT_NAMES_F32], axis=1))
    return packed


_NC_CACHE = {}


def get_nc():
    if "nc" not in _NC_CACHE:
        _NC_CACHE["nc"] = build_nc()
    return _NC_CACHE["nc"]


def make_in_maps(x0, x1, pkg):
    in_maps = []
    for core in range(8):
        b, half = core // 2, core % 2
        im = dict(pkg)
        for m, x in ((0, x0), (1, x1)):
            xi = x[b]
            if half == 1:
                xi = np.roll(xi, -T, axis=0)
            im[f"xT{m}"] = np.ascontiguousarray(xi.T.astype(bf16))
        in_maps.append(im)
    return in_maps


def assemble(results):
    out0 = np.empty((B, NIMG, C), np.float32)
    out1 = np.empty((B, NIMG, C), np.float32)
    for core in range(8):
        b, half = core // 2, core % 2
        o = results[core]["out"]               # [2, C, T] bf16
        sl = slice(0, T) if half == 0 else slice(T, NIMG)
        out0[b, sl] = o[0].T.astype(np.float32)
        out1[b, sl] = o[1].T.astype(np.float32)
    return out0, out1


def kernel(**inputs):
    x0 = _np(inputs["x0"]).astype(np.float32)
    x1 = _np(inputs["x1"]).astype(np.float32)
    pkg = prep_weights(inputs)
    nc = get_nc()
    in_maps = make_in_maps(x0, x1, pkg)
    res = run_bass_kernel_spmd(nc, in_maps, core_ids=list(range(8)))
    return assemble(res.results)
